# revision 14
# baseline (speedup 1.0000x reference)
"""GQA attention kernel for 8 Trainium2 NeuronCores.

Problem: B=2, N=2048, D=2048, H=32 heads, G=8 KV groups, head_dim=64, RoPE,
causal mask, fused QKV/output projections.

Sharding: one (batch, head-quarter) unit per core — core c handles batch c//4
and KV groups {2*(c%4), 2*(c%4)+1} (8 query heads). Each core computes a
partial output projection (its heads' rows of Wo); the host sums the 4
partials per batch.

Host-side prep (part of sharding): x is transposed and cast to bf16 in a
[ki=128, ko=16, tok] blocked layout so the device needs no transpose pass;
weights are similarly pre-blocked/cast.

Per-core pipeline (all matmuls bf16, fp32 accumulate):
  A: QKV projection (lhsT = xT blocks, combined [q|k|v] rhs), RoPE on DVE in
     natural layout, PE-transpose q/k to qT/kT [d, tok] (copies on ACT,
     which is otherwise idle in phase A); v stored natural with a ones
     column appended ([v|1]).
  B (per query-half hq of 1024): per head, key-block-major:
     scores^T[m] = kT_m.T @ qT (PSUM); causal diag-block mask applied by
     accumulating a -30000 upper-triangular matmul into the scores;
     exp on ACT -> attn^T (bf16 SBUF);
     ctx^T += [v_m|1].T @ attn^T  -> row 64 = softmax denominators.
     normalize: sums row DMA-reshaped to [128, 8] so the native DVE
     reciprocal runs partition-parallel, then DMA broadcast + multiply.
  C (per query-half): out = ctx^T.T @ Wo, bf16 partial written to DRAM.
     C(h=0) is emitted interleaved with B(h=1) heads so the PE fills the
     gaps of the ACT-bound attention phase.
"""

import numpy as np
import ml_dtypes

import concourse.bass as bass
import concourse.bacc as bacc
import concourse.mybir as mybir
import concourse.tile as tile
from concourse.bass_utils import run_bass_kernel_spmd
from concourse.masks import make_identity, make_upper_triangular

F32 = mybir.dt.float32
BF16 = mybir.dt.bfloat16

N = 2048          # sequence length
D = 2048          # model dim
HD = 64           # head dim
QF = 512          # q features per core (8 heads)
KF = 128          # k/v features per core (2 groups)
NT = N // 128     # token blocks
KC = D // 128     # contraction chunks
SCALE = 1.0 / 8.0  # 1/sqrt(HD)


def _build_program():
    nc = bacc.Bacc("TRN2", debug=False, target_bir_lowering=False)

    xT_d = nc.dram_tensor("xt", [128, KC, N], BF16, kind="ExternalInput")
    wqkv_d = nc.dram_tensor("wqkv", [128, KC, QF + 2 * KF], BF16,
                            kind="ExternalInput")
    wo_d = nc.dram_tensor("wo", [128, 4, D], BF16, kind="ExternalInput")
    cos_d = nc.dram_tensor("cos", [128, NT, HD], F32, kind="ExternalInput")
    sin_d = nc.dram_tensor("sin", [128, NT, HD], F32, kind="ExternalInput")
    out_d = nc.dram_tensor("out", [N, D], BF16, kind="ExternalOutput")

    with tile.TileContext(nc) as tc:
        with tc.tile_pool(name="persist", bufs=1) as pp:
            qT = [pp.tile([128, N], BF16, name=f"qT{t}") for t in range(4)]
            kT = pp.tile([128, N], BF16, name="kT")
            vo = [pp.tile([128, NT, HD + 1], BF16, name=f"vo{g}")
                  for g in range(2)]
            ctxT = [pp.tile([128, N], BF16, name=f"ctxT{k}") for k in range(4)]
            wo_sb = pp.tile([128, 4, D], BF16, name="wo_sb")
            ident = pp.tile([128, 128], BF16, name="ident")
            negtri = pp.tile([128, 128], BF16, name="negtri")

            make_identity(nc, ident)
            # strict upper triangle = -30000 (additive causal mask, applied
            # on the PE: psc += negtri.T @ ident puts -30000 at k > q)
            make_upper_triangular(nc, negtri, val=-30000.0, diag=False)
            for g in range(2):
                nc.vector.memset(vo[g][:, :, HD:HD + 1], 1.0)

            # ---------------- phase A: projections + rope ----------------
            with tc.tile_pool(name="phaseA", bufs=1) as pa, \
                 tc.tile_pool(name="ps_a", bufs=2, space="PSUM") as ps_a, \
                 tc.tile_pool(name="ps_tr", bufs=2, space="PSUM") as ps_tr, \
                 tc.tile_pool(name="ropetmp", bufs=6) as rtp:

                xT = pa.tile([128, KC, N], BF16, name="xT")
                wqkv_sb = pa.tile([128, KC, QF + 2 * KF], BF16, name="wqkv_sb")
                cos_sb = pa.tile([128, NT, HD], F32, name="cos_sb")
                sin_sb = pa.tile([128, NT, HD], F32, name="sin_sb")
                q_rope = pa.tile([128, NT, QF], BF16, name="q_rope")
                k_rope = pa.tile([128, NT, KF], BF16, name="k_rope")

                nc.gpsimd.dma_start(wqkv_sb[:], wqkv_d[:])
                nc.scalar.dma_start(cos_sb[:], cos_d[:])
                nc.scalar.dma_start(sin_sb[:], sin_d[:])
                # x^T arrives in 4 column chunks so compute can start early
                for ch in range(4):
                    cs_ = slice(ch * 512, (ch + 1) * 512)
                    nc.sync.dma_start(xT[:, :, cs_], xT_d[:, :, cs_])
                nc.gpsimd.dma_start(wo_sb[:], wo_d[:])

                def rope(ps, cos_b, sin_b, out_v, ab_shape):
                    """ps 4D view [128, *ab, 2, 32]; cos_b/sin_b broadcast
                    [128, *ab, 32]; out_v same 4D view layout as ps."""
                    q1 = ps[..., 0, :]
                    q2 = ps[..., 1, :]
                    c1, c2 = cos_b
                    s1, s2 = sin_b
                    ta = rtp.tile([128] + ab_shape + [32], F32, name="rt",
                                  tag="rt")
                    tb = rtp.tile([128] + ab_shape + [32], F32, name="rt",
                                  tag="rt")
                    nc.vector.tensor_mul(ta[:], q1, c1)
                    nc.vector.tensor_mul(tb[:], q2, s1)
                    nc.vector.tensor_sub(out_v[..., 0, :], ta[:], tb[:])
                    tc_ = rtp.tile([128] + ab_shape + [32], F32, name="rt",
                                   tag="rt")
                    td = rtp.tile([128] + ab_shape + [32], F32, name="rt",
                                  tag="rt")
                    nc.vector.tensor_mul(tc_[:], q2, c2)
                    nc.vector.tensor_mul(td[:], q1, s2)
                    nc.vector.tensor_add(out_v[..., 1, :], tc_[:], td[:])

                for tb_i in range(NT):
                    tcols = slice(tb_i * 128, (tb_i + 1) * 128)
                    psqkv = ps_a.tile([128, QF + 2 * KF], F32, name="psqkv",
                                      tag="psqkv")
                    for kc in range(KC):
                        lhsT = xT[:, kc, tcols]
                        st = kc == 0
                        sp = kc == KC - 1
                        nc.tensor.matmul(psqkv[:, 0:512], lhsT,
                                         wqkv_sb[:, kc, 0:512],
                                         start=st, stop=sp)
                        nc.tensor.matmul(psqkv[:, 512:768], lhsT,
                                         wqkv_sb[:, kc, 512:768],
                                         start=st, stop=sp)

                    # --- RoPE Q: psqkv q cols = a*256 + b*64 + h*32 + j
                    #     out cols = b*128 + a*64 + h*32 + j (head pairs
                    #     adjacent for the transpose step)
                    psq_v = psqkv[:, 0:QF].rearrange(
                        "p (a b h j) -> p a b h j", a=2, b=4, h=2)
                    out_v = q_rope[:, tb_i, :].rearrange(
                        "p (b a h j) -> p a b h j", b=4, a=2, h=2)
                    cs = cos_sb[:, tb_i, :]
                    sn = sin_sb[:, tb_i, :]

                    def bcq(apv):
                        return apv.unsqueeze(1).unsqueeze(1).broadcast_to(
                            (128, 2, 4, 32))

                    rope(psq_v,
                         (bcq(cs[:, 0:32]), bcq(cs[:, 32:64])),
                         (bcq(sn[:, 0:32]), bcq(sn[:, 32:64])),
                         out_v, [2, 4])

                    # --- RoPE K: cols = g*64 + h*32 + j (no interleave)
                    psk_v = psqkv[:, QF:QF + KF].rearrange(
                        "p (g h j) -> p g h j", g=2, h=2)
                    outk_v = k_rope[:, tb_i, :].rearrange(
                        "p (g h j) -> p g h j", g=2, h=2)

                    def bck(apv):
                        return apv.unsqueeze(1).broadcast_to((128, 2, 32))

                    rope(psk_v,
                         (bck(cs[:, 0:32]), bck(cs[:, 32:64])),
                         (bck(sn[:, 0:32]), bck(sn[:, 32:64])),
                         outk_v, [2])

                    # --- V -> bf16 SBUF with ones column (ACT: idle in A)
                    for g in range(2):
                        nc.scalar.copy(
                            vo[g][:, tb_i, 0:HD],
                            psqkv[:, QF + KF + g * 64:QF + KF + (g + 1) * 64])

                    # --- PE transposes: q_rope/k_rope -> qT/kT (copies on
                    #     ACT, idle during phase A)
                    for t in range(4):
                        ptr = ps_tr.tile([128, 128], BF16, name="ptr",
                                         tag="ptr")
                        nc.tensor.transpose(
                            ptr[:], q_rope[:, tb_i, t * 128:(t + 1) * 128],
                            ident[:])
                        nc.scalar.copy(qT[t][:, tcols], ptr[:])
                    ptrk = ps_tr.tile([128, 128], BF16, name="ptr", tag="ptr")
                    nc.tensor.transpose(ptrk[:], k_rope[:, tb_i, :], ident[:])
                    nc.scalar.copy(kT[:, tcols], ptrk[:])

            # ------------- phase B+C: attention + out-proj per half -------
            with tc.tile_pool(name="ps_big", bufs=2, space="PSUM") as psb, \
                 tc.tile_pool(name="ps_sc", bufs=2, space="PSUM") as ps_sc, \
                 tc.tile_pool(name="attnp", bufs=6) as ap_, \
                 tc.tile_pool(name="dramn", bufs=4, space="DRAM") as dnp, \
                 tc.tile_pool(name="normp", bufs=2) as np_, \
                 tc.tile_pool(name="outp", bufs=3) as op_:

                def normalize(h, l, psx):
                    """sums row -> [128, 8] via DRAM reshape so the native
                    reciprocal runs partition-parallel; broadcast + mul."""
                    h0 = 1024 * h
                    hcols = slice(h0, h0 + 1024)
                    rrow = np_.tile([65, 1024], F32, name="rrow", tag="rrow")
                    nc.vector.tensor_copy(rrow[64:65, :], psx[64:65, :])
                    rd1 = dnp.tile([1, 1024], F32, name="rd1", tag="rd1")
                    nc.sync.dma_start(rd1[:], rrow[64:65, :])
                    rcol = np_.tile([128, 8], F32, name="rcol", tag="rcol")
                    nc.sync.dma_start(
                        rcol[:], rd1[:].rearrange("o (p j) -> (o p) j", p=128))
                    rcr = np_.tile([128, 8], F32, name="rcr", tag="rcr")
                    nc.vector.reciprocal(rcr[:], rcol[:])
                    rd2 = dnp.tile([1, 1024], F32, name="rd2", tag="rd2")
                    nc.sync.dma_start(
                        rd2[:].rearrange("o (p j) -> (o p) j", p=128), rcr[:])
                    rb = np_.tile([64, 1024], F32, name="rb", tag="rb")
                    nc.sync.dma_start(rb[:], rd2[:].to_broadcast((64, 1024)))
                    pk = l // 2
                    if l % 2 == 0:
                        nc.vector.tensor_mul(ctxT[pk][0:64, hcols],
                                             psx[0:64, :], rb[:])
                    else:
                        codd = np_.tile([64, 1024], BF16, name="codd",
                                        tag="codd")
                        nc.vector.tensor_mul(codd[:], psx[0:64, :], rb[:])
                        nc.sync.dma_start(ctxT[pk][64:128, hcols], codd[:])

                def attn_pair(h, b):
                    """Heads (b, b+4) pipelined together: per key block the
                    two K=64 score matmuls co-run in disjoint PE row groups
                    (rows 0-63 / 64-127), and one head's exp overlaps the
                    other head's PE work."""
                    h0 = 1024 * h
                    mlast = 8 * (h + 1) - 1
                    psx = [psb.tile([128, 1024], F32, name="psx", tag="big")
                           for _ in range(2)]
                    for m in range(mlast + 1):
                        qlo = max(h0, 128 * m)
                        w = h0 + 1024 - qlo
                        mcols = slice(m * 128, (m + 1) * 128)
                        psc = []
                        ats = []
                        for a in range(2):
                            r0 = 64 * a
                            p = ps_sc.tile([128, 1024], F32, name="psc",
                                           tag="psc")
                            psc.append(p)
                            has_diag = 128 * m >= h0
                            off = 0
                            while off < w:
                                nw = min(512, w - off)
                                nc.tensor.matmul(
                                    p[:, off:off + nw],
                                    kT[r0:r0 + 64, mcols],
                                    qT[b][r0:r0 + 64,
                                          qlo + off:qlo + off + nw],
                                    start=True,
                                    stop=(off + nw >= w and not has_diag),
                                    skip_group_check=True)
                                off += nw
                        for a in range(2):
                            if 128 * m >= h0:
                                # diag block: additive causal mask on the PE
                                nc.tensor.matmul(
                                    psc[a][:, 0:128], negtri[:], ident[:],
                                    start=False, stop=True,
                                    skip_group_check=True)
                            at = ap_.tile([128, 1024], BF16, name="at",
                                          tag="at")
                            ats.append(at)
                            nc.scalar.activation(
                                at[:, :w], psc[a][:, :w],
                                mybir.ActivationFunctionType.Exp, scale=SCALE)
                        # ctx^T accumulation, chunks within psum banks
                        for a in range(2):
                            qoff0 = qlo - h0
                            off = 0
                            while off < w:
                                gc = qoff0 + off
                                nw = min(512 - gc % 512, w - off)
                                m_true = min(mlast,
                                             (h0 + gc + nw - 1) // 128)
                                nc.tensor.matmul(
                                    psx[a][0:65, gc:gc + nw], vo[a][:, m, :],
                                    ats[a][:, off:off + nw],
                                    start=(m == 0), stop=(m == m_true),
                                    skip_group_check=True)
                                off += nw
                    normalize(h, b, psx[0])
                    normalize(h, b + 4, psx[1])

                def proj_tb(tb_i, engines):
                    """phase C for one token block: out = ctx^T.T @ Wo."""
                    tcols = slice(tb_i * 128, (tb_i + 1) * 128)
                    for dh in range(2):
                        pso = psb.tile([128, 1024], F32, name="pso",
                                       tag="big")
                        for k4 in range(4):
                            lhsT = ctxT[k4][:, tcols]
                            for j in range(2):
                                nc.tensor.matmul(
                                    pso[:, j * 512:(j + 1) * 512], lhsT,
                                    wo_sb[:, k4,
                                          dh * 1024 + j * 512:
                                          dh * 1024 + (j + 1) * 512],
                                    start=(k4 == 0), stop=(k4 == 3))
                        ost = op_.tile([128, 1024], BF16, name="ost",
                                       tag="ost")
                        eng = engines[dh]
                        if eng == "v":
                            nc.vector.tensor_copy(ost[:], pso[:])
                        else:
                            nc.scalar.copy(ost[:], pso[:])
                        nc.sync.dma_start(
                            out_d[tcols, dh * 1024:(dh + 1) * 1024], ost[:])

                for b in range(4):
                    attn_pair(0, b)
                for b in range(4):
                    attn_pair(1, b)
                    # C(h0) rides under B(h1)'s ACT-bound pairs
                    proj_tb(2 * b, "vv")
                    proj_tb(2 * b + 1, "vv")
                for tb_i in range(8, 16):
                    proj_tb(tb_i, "vs")    # tail: split copies ACT/DVE

    nc.compile()
    return nc


_NC_CACHE = {}


def _get_nc():
    if "nc" not in _NC_CACHE:
        _NC_CACHE["nc"] = _build_program()
    return _NC_CACHE["nc"]


def _block(arr, ko):
    """[ko*128, cols] -> [128, ko, cols] blocked bf16 layout."""
    a = np.asarray(arr, dtype=np.float32).reshape(ko, 128, arr.shape[-1])
    return np.ascontiguousarray(a.transpose(1, 0, 2)).astype(ml_dtypes.bfloat16)


def kernel(x, cos, sin, mask, Wq, Wk, Wv, Wo, _trace=False, _trace_kwargs=None):
    x = np.asarray(x, dtype=np.float32)
    cos = np.asarray(cos, dtype=np.float32)
    sin = np.asarray(sin, dtype=np.float32)
    Wq = np.asarray(Wq, dtype=np.float32)
    Wk = np.asarray(Wk, dtype=np.float32)
    Wv = np.asarray(Wv, dtype=np.float32)
    Wo = np.asarray(Wo, dtype=np.float32)

    nc = _get_nc()

    # host-side sharding prep
    xT_b = [_block(np.ascontiguousarray(x[bi].T), KC) for bi in range(2)]
    cos_b = np.ascontiguousarray(
        cos.reshape(NT, 128, HD).transpose(1, 0, 2)).astype(np.float32)
    sin_b = np.ascontiguousarray(
        sin.reshape(NT, 128, HD).transpose(1, 0, 2)).astype(np.float32)

    in_maps = []
    for c in range(8):
        bi = c // 4
        p = c % 4
        wqkv = np.concatenate([
            Wq[:, p * QF:(p + 1) * QF],
            Wk[:, p * KF:(p + 1) * KF],
            Wv[:, p * KF:(p + 1) * KF],
        ], axis=1)
        wo_p = Wo[p * QF:(p + 1) * QF, :]
        in_maps.append({
            "xt": xT_b[bi],
            "wqkv": _block(wqkv, KC),
            "wo": _block(wo_p, 4),
            "cos": cos_b,
            "sin": sin_b,
        })

    kwargs = {}
    if _trace:
        kwargs["trace"] = True
        kwargs.update(_trace_kwargs or {})
    res = run_bass_kernel_spmd(nc, in_maps, core_ids=list(range(8)), **kwargs)
    parts = [r["out"].astype(np.float32) for r in res.results]
    out = np.stack([
        parts[0] + parts[1] + parts[2] + parts[3],
        parts[4] + parts[5] + parts[6] + parts[7],
    ]).astype(np.float32)
    if _trace:
        kernel._last_result = res
    return out


# revision 17
# speedup vs baseline: 1.0844x; 1.0844x over previous
"""GQA attention kernel for 8 Trainium2 NeuronCores.

Problem: B=2, N=2048, D=2048, H=32 heads, G=8 KV groups, head_dim=64, RoPE,
causal mask, fused QKV/output projections.

Sharding: one (batch, head-quarter) unit per core — core c handles batch c//4
and KV groups {2*(c%4), 2*(c%4)+1} (8 query heads). Each core computes a
partial output projection (its heads' rows of Wo); the host sums the 4
partials per batch.

Host-side prep (part of sharding): x is transposed and cast to bf16 in a
[ki=128, ko=16, tok] blocked layout so the device needs no transpose pass;
weights are similarly pre-blocked/cast.

Per-core pipeline (all matmuls bf16, fp32 accumulate):
  A: QKV projection (lhsT = xT blocks, combined [q|k|v] rhs), RoPE on DVE in
     natural layout, PE-transpose q/k to qT/kT [d, tok] (copies on ACT,
     which is otherwise idle in phase A); v stored natural with a ones
     column appended ([v|1]).
  B (per query-half hq of 1024): per head, key-block-major:
     scores^T[m] = kT_m.T @ qT (PSUM); causal diag-block mask applied by
     accumulating a -30000 upper-triangular matmul into the scores;
     exp on ACT -> attn^T (bf16 SBUF);
     ctx^T += [v_m|1].T @ attn^T  -> row 64 = softmax denominators.
     normalize: sums row DMA-reshaped to [128, 8] so the native DVE
     reciprocal runs partition-parallel, then DMA broadcast + multiply.
  C (per query-half): out = ctx^T.T @ Wo, bf16 partial written to DRAM.
     C(h=0) is emitted interleaved with B(h=1) heads so the PE fills the
     gaps of the ACT-bound attention phase.
"""

import numpy as np
import ml_dtypes

import concourse.bass as bass
import concourse.bacc as bacc
import concourse.mybir as mybir
import concourse.tile as tile
from concourse.bass_utils import run_bass_kernel_spmd
from concourse.masks import make_identity, make_upper_triangular

F32 = mybir.dt.float32
BF16 = mybir.dt.bfloat16

N = 2048          # sequence length
D = 2048          # model dim
HD = 64           # head dim
QF = 512          # q features per core (8 heads)
KF = 128          # k/v features per core (2 groups)
NT = N // 128     # token blocks
KC = D // 128     # contraction chunks
SCALE = 1.0 / 8.0  # 1/sqrt(HD)


def _build_program():
    nc = bacc.Bacc("TRN2", debug=False, target_bir_lowering=False)

    xT_d = nc.dram_tensor("xt", [128, KC, N], BF16, kind="ExternalInput")
    wqkv_d = nc.dram_tensor("wqkv", [128, KC, QF + 2 * KF], BF16,
                            kind="ExternalInput")
    wo_d = nc.dram_tensor("wo", [128, 4, D], BF16, kind="ExternalInput")
    cos_d = nc.dram_tensor("cos", [128, NT, HD], F32, kind="ExternalInput")
    sin_d = nc.dram_tensor("sin", [128, NT, HD], F32, kind="ExternalInput")
    out_d = nc.dram_tensor("out", [N, D], BF16, kind="ExternalOutput")

    with tile.TileContext(nc) as tc:
        with tc.tile_pool(name="persist", bufs=1) as pp:
            qT = [pp.tile([128, N], BF16, name=f"qT{t}") for t in range(4)]
            kT = pp.tile([128, N], BF16, name="kT")
            vo = [pp.tile([128, NT, HD + 1], BF16, name=f"vo{g}")
                  for g in range(2)]
            ctxT = [pp.tile([128, N], BF16, name=f"ctxT{k}") for k in range(4)]
            wo_sb = pp.tile([128, 4, D], BF16, name="wo_sb")
            ident = pp.tile([128, 128], BF16, name="ident")
            negtri = pp.tile([128, 128], BF16, name="negtri")

            make_identity(nc, ident)
            # strict upper triangle = -30000 (additive causal mask, applied
            # on the PE: psc += negtri.T @ ident puts -30000 at k > q)
            make_upper_triangular(nc, negtri, val=-30000.0, diag=False)
            for g in range(2):
                nc.vector.memset(vo[g][:, :, HD:HD + 1], 1.0)

            # ---------------- phase A: projections + rope ----------------
            with tc.tile_pool(name="phaseA", bufs=1) as pa, \
                 tc.tile_pool(name="ps_a", bufs=2, space="PSUM") as ps_a, \
                 tc.tile_pool(name="ps_tr", bufs=2, space="PSUM") as ps_tr, \
                 tc.tile_pool(name="ropetmp", bufs=6) as rtp:

                xT = pa.tile([128, KC, N], BF16, name="xT")
                wqkv_sb = pa.tile([128, KC, QF + 2 * KF], BF16, name="wqkv_sb")
                cos_sb = pa.tile([128, NT, HD], F32, name="cos_sb")
                sin_sb = pa.tile([128, NT, HD], F32, name="sin_sb")
                q_rope = pa.tile([128, NT, QF], BF16, name="q_rope")
                k_rope = pa.tile([128, NT, KF], BF16, name="k_rope")

                nc.gpsimd.dma_start(wqkv_sb[:], wqkv_d[:])
                nc.scalar.dma_start(cos_sb[:], cos_d[:])
                nc.scalar.dma_start(sin_sb[:], sin_d[:])
                # x^T arrives in 4 column chunks, split over the sync and
                # scalar queues, so compute can start early
                for ch in range(4):
                    cs_ = slice(ch * 512, (ch + 1) * 512)
                    q_ = nc.sync if ch % 2 == 0 else nc.scalar
                    q_.dma_start(xT[:, :, cs_], xT_d[:, :, cs_])
                nc.gpsimd.dma_start(wo_sb[:], wo_d[:])

                def rope(ps, cos_b, sin_b, out_v, ab_shape):
                    """ps 4D view [128, *ab, 2, 32]; cos_b/sin_b broadcast
                    [128, *ab, 32]; out_v same 4D view layout as ps."""
                    q1 = ps[..., 0, :]
                    q2 = ps[..., 1, :]
                    c1, c2 = cos_b
                    s1, s2 = sin_b
                    ta = rtp.tile([128] + ab_shape + [32], F32, name="rt",
                                  tag="rt")
                    tb = rtp.tile([128] + ab_shape + [32], F32, name="rt",
                                  tag="rt")
                    nc.vector.tensor_mul(ta[:], q1, c1)
                    nc.vector.tensor_mul(tb[:], q2, s1)
                    nc.vector.tensor_sub(out_v[..., 0, :], ta[:], tb[:])
                    tc_ = rtp.tile([128] + ab_shape + [32], F32, name="rt",
                                   tag="rt")
                    td = rtp.tile([128] + ab_shape + [32], F32, name="rt",
                                  tag="rt")
                    nc.vector.tensor_mul(tc_[:], q2, c2)
                    nc.vector.tensor_mul(td[:], q1, s2)
                    nc.vector.tensor_add(out_v[..., 1, :], tc_[:], td[:])

                for tb_i in range(NT):
                    tcols = slice(tb_i * 128, (tb_i + 1) * 128)
                    psqkv = ps_a.tile([128, QF + 2 * KF], F32, name="psqkv",
                                      tag="psqkv")
                    for kc in range(KC):
                        lhsT = xT[:, kc, tcols]
                        st = kc == 0
                        sp = kc == KC - 1
                        nc.tensor.matmul(psqkv[:, 0:512], lhsT,
                                         wqkv_sb[:, kc, 0:512],
                                         start=st, stop=sp)
                        nc.tensor.matmul(psqkv[:, 512:768], lhsT,
                                         wqkv_sb[:, kc, 512:768],
                                         start=st, stop=sp)

                    # --- RoPE Q: psqkv q cols = a*256 + b*64 + h*32 + j
                    #     out cols = b*128 + a*64 + h*32 + j (head pairs
                    #     adjacent for the transpose step)
                    psq_v = psqkv[:, 0:QF].rearrange(
                        "p (a b h j) -> p a b h j", a=2, b=4, h=2)
                    out_v = q_rope[:, tb_i, :].rearrange(
                        "p (b a h j) -> p a b h j", b=4, a=2, h=2)
                    cs = cos_sb[:, tb_i, :]
                    sn = sin_sb[:, tb_i, :]

                    def bcq(apv):
                        return apv.unsqueeze(1).unsqueeze(1).broadcast_to(
                            (128, 2, 4, 32))

                    rope(psq_v,
                         (bcq(cs[:, 0:32]), bcq(cs[:, 32:64])),
                         (bcq(sn[:, 0:32]), bcq(sn[:, 32:64])),
                         out_v, [2, 4])

                    # --- RoPE K: cols = g*64 + h*32 + j (no interleave)
                    psk_v = psqkv[:, QF:QF + KF].rearrange(
                        "p (g h j) -> p g h j", g=2, h=2)
                    outk_v = k_rope[:, tb_i, :].rearrange(
                        "p (g h j) -> p g h j", g=2, h=2)

                    def bck(apv):
                        return apv.unsqueeze(1).broadcast_to((128, 2, 32))

                    rope(psk_v,
                         (bck(cs[:, 0:32]), bck(cs[:, 32:64])),
                         (bck(sn[:, 0:32]), bck(sn[:, 32:64])),
                         outk_v, [2])

                    # --- V -> bf16 SBUF with ones column (ACT: idle in A)
                    for g in range(2):
                        nc.scalar.copy(
                            vo[g][:, tb_i, 0:HD],
                            psqkv[:, QF + KF + g * 64:QF + KF + (g + 1) * 64])

                    # --- PE transposes: q_rope/k_rope -> qT/kT (copies on
                    #     ACT, idle during phase A)
                    for t in range(4):
                        ptr = ps_tr.tile([128, 128], BF16, name="ptr",
                                         tag="ptr")
                        nc.tensor.transpose(
                            ptr[:], q_rope[:, tb_i, t * 128:(t + 1) * 128],
                            ident[:])
                        nc.scalar.copy(qT[t][:, tcols], ptr[:])
                    ptrk = ps_tr.tile([128, 128], BF16, name="ptr", tag="ptr")
                    nc.tensor.transpose(ptrk[:], k_rope[:, tb_i, :], ident[:])
                    nc.scalar.copy(kT[:, tcols], ptrk[:])

            # ------------- phase B+C: attention + out-proj per half -------
            with tc.tile_pool(name="ps_big", bufs=2, space="PSUM") as psb, \
                 tc.tile_pool(name="ps_sc", bufs=2, space="PSUM") as ps_sc, \
                 tc.tile_pool(name="attnp", bufs=6) as ap_, \
                 tc.tile_pool(name="dramn", bufs=4, space="DRAM") as dnp, \
                 tc.tile_pool(name="normp", bufs=2) as np_, \
                 tc.tile_pool(name="outp", bufs=2) as op_:

                def attn_head(h, l):
                    """scores + exp + ctx + normalize for head l, half h."""
                    h0 = 1024 * h
                    hcols = slice(h0, h0 + 1024)
                    mlast = 8 * (h + 1) - 1
                    a, b = l // 4, l % 4
                    r0 = 64 * a
                    psx = psb.tile([128, 1024], F32, name="psx", tag="big")
                    for m in range(mlast + 1):
                        qlo = max(h0, 128 * m)
                        w = h0 + 1024 - qlo
                        lhs_k = kT[r0:r0 + 64, m * 128:(m + 1) * 128]
                        psc = ps_sc.tile([128, 1024], F32, name="psc",
                                         tag="psc")
                        off = 0
                        while off < w:
                            nw = min(512, w - off)
                            nc.tensor.matmul(
                                psc[:, off:off + nw], lhs_k,
                                qT[b][r0:r0 + 64, qlo + off:qlo + off + nw],
                                start=True, stop=(off + nw >= w or m < 8 * h),
                                skip_group_check=True)
                            off += nw
                        if 128 * m >= h0:
                            # diagonal block: additive causal mask on the PE
                            nc.tensor.matmul(
                                psc[:, 0:128], negtri[:], ident[:],
                                start=False, stop=True, skip_group_check=True)
                        at = ap_.tile([128, 1024], BF16, name="at", tag="at")
                        nc.scalar.activation(
                            at[:, :w], psc[:, :w],
                            mybir.ActivationFunctionType.Exp, scale=SCALE)
                        # ctx^T accumulation, chunks within psum banks
                        qoff0 = qlo - h0
                        off = 0
                        while off < w:
                            gc = qoff0 + off
                            nw = min(512 - gc % 512, w - off)
                            m_true = min(mlast, (h0 + gc + nw - 1) // 128)
                            nc.tensor.matmul(
                                psx[0:65, gc:gc + nw], vo[a][:, m, :],
                                at[:, off:off + nw],
                                start=(m == 0), stop=(m == m_true),
                                skip_group_check=True)
                            off += nw

                    # normalize: sums row -> [128, 8] via DRAM reshape so the
                    # native reciprocal runs partition-parallel
                    rrow = np_.tile([65, 1024], F32, name="rrow", tag="rrow")
                    nc.vector.tensor_copy(rrow[64:65, :], psx[64:65, :])
                    rd1 = dnp.tile([1, 1024], F32, name="rd1", tag="rd1")
                    nc.sync.dma_start(rd1[:], rrow[64:65, :])
                    rcol = np_.tile([128, 8], F32, name="rcol", tag="rcol")
                    nc.sync.dma_start(
                        rcol[:], rd1[:].rearrange("o (p j) -> (o p) j", p=128))
                    rcr = np_.tile([128, 8], F32, name="rcr", tag="rcr")
                    nc.vector.reciprocal(rcr[:], rcol[:])
                    rd2 = dnp.tile([1, 1024], F32, name="rd2", tag="rd2")
                    nc.sync.dma_start(
                        rd2[:].rearrange("o (p j) -> (o p) j", p=128), rcr[:])
                    rb = np_.tile([64, 1024], F32, name="rb", tag="rb")
                    nc.sync.dma_start(rb[:], rd2[:].to_broadcast((64, 1024)))
                    pk = l // 2
                    if l % 2 == 0:
                        nc.vector.tensor_mul(ctxT[pk][0:64, hcols],
                                             psx[0:64, :], rb[:])
                    else:
                        codd = np_.tile([64, 1024], BF16, name="codd",
                                        tag="codd")
                        nc.vector.tensor_mul(codd[:], psx[0:64, :], rb[:])
                        nc.sync.dma_start(ctxT[pk][64:128, hcols], codd[:])

                def proj_tb(tb_i, engines):
                    """phase C for one token block: out = ctx^T.T @ Wo."""
                    tcols = slice(tb_i * 128, (tb_i + 1) * 128)
                    for dh in range(2):
                        pso = psb.tile([128, 1024], F32, name="pso",
                                       tag="big")
                        for k4 in range(4):
                            lhsT = ctxT[k4][:, tcols]
                            for j in range(2):
                                nc.tensor.matmul(
                                    pso[:, j * 512:(j + 1) * 512], lhsT,
                                    wo_sb[:, k4,
                                          dh * 1024 + j * 512:
                                          dh * 1024 + (j + 1) * 512],
                                    start=(k4 == 0), stop=(k4 == 3))
                        ost = op_.tile([128, 1024], BF16, name="ost",
                                       tag="ost")
                        eng = engines[dh]
                        if eng == "v":
                            nc.vector.tensor_copy(ost[:], pso[:])
                        else:
                            nc.scalar.copy(ost[:], pso[:])
                        nc.sync.dma_start(
                            out_d[tcols, dh * 1024:(dh + 1) * 1024], ost[:])

                for l in range(8):
                    attn_head(0, l)
                for l in range(8):
                    attn_head(1, l)
                    proj_tb(l, "vv")       # C(h0) rides under B(h1)'s ACT
                for tb_i in range(8, 16):
                    proj_tb(tb_i, "vs")    # tail: split copies ACT/DVE

    nc.compile()
    return nc


_NC_CACHE = {}


def _get_nc():
    if "nc" not in _NC_CACHE:
        _NC_CACHE["nc"] = _build_program()
    return _NC_CACHE["nc"]


def _block(arr, ko):
    """[ko*128, cols] -> [128, ko, cols] blocked bf16 layout."""
    a = np.asarray(arr, dtype=np.float32).reshape(ko, 128, arr.shape[-1])
    return np.ascontiguousarray(a.transpose(1, 0, 2)).astype(ml_dtypes.bfloat16)


def kernel(x, cos, sin, mask, Wq, Wk, Wv, Wo, _trace=False, _trace_kwargs=None):
    x = np.asarray(x, dtype=np.float32)
    cos = np.asarray(cos, dtype=np.float32)
    sin = np.asarray(sin, dtype=np.float32)
    Wq = np.asarray(Wq, dtype=np.float32)
    Wk = np.asarray(Wk, dtype=np.float32)
    Wv = np.asarray(Wv, dtype=np.float32)
    Wo = np.asarray(Wo, dtype=np.float32)

    nc = _get_nc()

    # host-side sharding prep
    xT_b = [_block(np.ascontiguousarray(x[bi].T), KC) for bi in range(2)]
    cos_b = np.ascontiguousarray(
        cos.reshape(NT, 128, HD).transpose(1, 0, 2)).astype(np.float32)
    sin_b = np.ascontiguousarray(
        sin.reshape(NT, 128, HD).transpose(1, 0, 2)).astype(np.float32)

    in_maps = []
    for c in range(8):
        bi = c // 4
        p = c % 4
        wqkv = np.concatenate([
            Wq[:, p * QF:(p + 1) * QF],
            Wk[:, p * KF:(p + 1) * KF],
            Wv[:, p * KF:(p + 1) * KF],
        ], axis=1)
        wo_p = Wo[p * QF:(p + 1) * QF, :]
        in_maps.append({
            "xt": xT_b[bi],
            "wqkv": _block(wqkv, KC),
            "wo": _block(wo_p, 4),
            "cos": cos_b,
            "sin": sin_b,
        })

    kwargs = {}
    if _trace:
        kwargs["trace"] = True
        kwargs.update(_trace_kwargs or {})
    res = run_bass_kernel_spmd(nc, in_maps, core_ids=list(range(8)), **kwargs)
    parts = [r["out"].astype(np.float32) for r in res.results]
    out = np.stack([
        parts[0] + parts[1] + parts[2] + parts[3],
        parts[4] + parts[5] + parts[6] + parts[7],
    ]).astype(np.float32)
    if _trace:
        kernel._last_result = res
    return out
